# revision 1
# baseline (speedup 1.0000x reference)
"""Batch per-sample 3x3 conv (B=32, C=32, H=W=256, pad=1) on 8 TRN2 cores.

Data parallel: 4 samples per core, stacked on the 4 32-partition groups.
The per-sample convs run as 9 accumulating matmuls per PSUM bank (one per
3x3 tap) with a block-diagonal [128,128] stationary (4 samples on both the
contraction and output dims); spatial shifts are access-pattern offsets
into a sliding SBUF row window, with dx=+-1 taps written as partial-width
strided PSUM slices so the x DMAs stay fully contiguous. Inputs are cast
to fp16 on the host (10-bit mantissa keeps scale-relative error ~3e-4
while streaming the PE at its full 1 column/cycle); PSUM accumulates in
fp32 and ScalarE fuses the bias add into the PSUM drain.
"""

import numpy as np

N_CORES = 8
B, C_IN, C_OUT, H, W, KS = 32, 32, 32, 256, 256, 3
SPC = B // N_CORES  # samples per core
R = 64  # output rows per chunk
NCH = H // R
BANKS = R // 2  # psum banks per chunk (2 rows of 256 = 512 cols each)

DTYPE = "fp16"  # "fp16" | "f32r" | "bf16"

_CACHE = {}


def _build(dtype=DTYPE):
    import concourse.bacc as bacc
    import concourse.mybir as mybir
    import concourse.tile as tile

    f32 = mybir.dt.float32
    if dtype == "f32r":
        mm_dt, io_dt, ebytes = mybir.dt.float32r, f32, 4
    elif dtype == "fp16":
        mm_dt, io_dt, ebytes = mybir.dt.float16, mybir.dt.float16, 2
    else:
        mm_dt, io_dt, ebytes = mybir.dt.bfloat16, mybir.dt.bfloat16, 2

    nc = bacc.Bacc(
        "TRN2", target_bir_lowering=False, debug=False, num_devices=N_CORES
    )
    x_d = nc.dram_tensor("x", [128, H, W], io_dt, kind="ExternalInput").ap()
    w_d = nc.dram_tensor("w", [128, 9 * 128], io_dt, kind="ExternalInput").ap()
    bias_d = nc.dram_tensor("bias_v", [128, 1], f32, kind="ExternalInput").ap()
    o_d = nc.dram_tensor("out", [128, H, W], f32, kind="ExternalOutput").ap()
    if dtype == "f32r":
        x_d, w_d = x_d.bitcast(mm_dt), w_d.bitcast(mm_dt)

    with tile.TileContext(nc) as tc:
        with (
            tc.tile_pool(name="const", bufs=1) as cpool,
            tc.tile_pool(name="xp", bufs=1) as xpool,
            tc.tile_pool(name="op", bufs=2) as opool,
            tc.tile_pool(name="ps", bufs=8, space="PSUM") as ppool,
        ):
            # weights/bias go on the gpsimd queues so the sync-engine queues
            # are free for the first x pieces; tap 0 ships first
            w_sb = cpool.tile([128, 9 * 128], mm_dt)
            nc.gpsimd.dma_start(out=w_sb[:, 0:128], in_=w_d[:, 0:128])
            nc.gpsimd.dma_start(out=w_sb[:, 128:], in_=w_d[:, 128:])
            b_sb = cpool.tile([128, 1], f32)
            nc.gpsimd.dma_start(out=b_sb[:], in_=bias_d[:])

            u32 = mybir.dt.uint32 if ebytes == 4 else mybir.dt.uint16
            NXBUF = 2
            xbufs = [
                xpool.tile([128, R + 2, W], mm_dt, tag=f"xb{i}", name=f"xb{i}")
                for i in range(NXBUF)
            ]
            # top pad row for the first chunk (dy=-1 of output row 0)
            nc.vector.memset(xbufs[0][:, 0, :].bitcast(u32), 0)

            # tap order: dx=0 taps first so the start=True matmul covers the
            # full psum bank (dx=+-1 taps write partial-width strided slices)
            TAPS = [(dy, 0) for dy in (-1, 0, 1)] + [
                (dy, dx) for dx in (-1, 1) for dy in (-1, 0, 1)
            ]

            # warm the PE clock (HAM gate) during the initial x DMA wait:
            # dummy matmuls on a zeroed scratch tile, no data dependencies
            dumw = cpool.tile([128, 640], mm_dt)
            nc.vector.memset(dumw[:].bitcast(u32), 0)
            psw = ppool.tile([128, 2, W], f32, tag="ps0", name="psw", bufs=1)
            NWARM = 16
            for k in range(NWARM):
                nc.tensor.matmul(
                    psw[:, :, :],
                    dumw[:, 0:128],
                    dumw[:, 128:640],
                    start=(k == 0),
                    stop=(k == NWARM - 1),
                )

            for ch in range(NCH):
                r0 = ch * R
                xb = xbufs[ch % NXBUF]
                lo = max(r0 - 1, 0)
                hi = min(r0 + R + 1, H)
                dst0 = lo - (r0 - 1)
                # split the row-window DMA so compute can start on the first
                # rows while the rest streams in
                if ch == 0:
                    bounds = [0, 4]
                    while bounds[-1] < hi:
                        bounds.append(min(bounds[-1] + 12, hi))
                else:
                    t = (hi - lo) // 3
                    bounds = [lo, lo + t, lo + 2 * t, hi]
                for a, b in zip(bounds[:-1], bounds[1:]):
                    nc.sync.dma_start(
                        out=xb[:, dst0 + (a - lo) : dst0 + (b - lo), :],
                        in_=x_d[:, a:b, :],
                    )
                if hi < r0 + R + 1:  # bottom pad row for the last chunk
                    nc.vector.memset(xb[:, R + 1, :].bitcast(u32), 0)

                ob = opool.tile([128, R, W], f32, tag="ob", name="ob", bufs=1)
                # sweeps of 8 psum banks, taps outer so 8 consecutive
                # matmuls share the same stationary weights
                for sw in range(BANKS // 8):
                    pss = [
                        ppool.tile(
                            [128, 2, W], f32, tag=f"ps{j}", name=f"ps{j}", bufs=1
                        )
                        for j in range(8)
                    ]
                    last_sweep = ch == NCH - 1 and sw == BANKS // 8 - 1
                    for ti, j in [(t, j) for t in range(9) for j in range(8)]:
                        dy, dx = TAPS[ti]
                        tap = (dy + 1) * 3 + (dx + 1)
                        # out col w <- x col w+dx; clip to the image border
                        xa, ow = max(dx, 0), max(-dx, 0)
                        n = W - abs(dx)
                        bk = sw * 8 + j
                        row = 2 * bk + 1 + dy
                        nc.tensor.matmul(
                            pss[j][:, :, ow : ow + n],
                            w_sb[:, tap * 128 : (tap + 1) * 128],
                            xb[:, row : row + 2, xa : xa + n],
                            start=(ti == 0),
                            stop=(ti == 8),
                        )
                    if last_sweep:  # drain the tail in ever-smaller pieces
                        b = BANKS - 8
                        flush_at = {b+1: b, b+3: b+2, b+5: b+4, b+6: b+6, b+7: b+7}
                    else:
                        flush_at = {sw * 8 + 3: sw * 8, sw * 8 + 7: sw * 8 + 4}
                    for j in range(8):
                        bk = sw * 8 + j
                        if last_sweep and j % 2 == 1:
                            # split the final drain burst across two engines
                            nc.vector.tensor_scalar_add(
                                ob[:, 2 * bk : 2 * bk + 2, :],
                                pss[j][:, :, :],
                                b_sb[:, :],
                            )
                        else:
                            nc.scalar.add(
                                out=ob[:, 2 * bk : 2 * bk + 2, :],
                                in_=pss[j][:, :, :],
                                add=b_sb[:, :],
                            )
                        if bk in flush_at:
                            b0 = flush_at[bk]
                            rr = r0 + 2 * b0
                            nc.sync.dma_start(
                                out=o_d[:, rr : rr + 2 * (bk - b0 + 1), :],
                                in_=ob[:, 2 * b0 : 2 * (bk + 1), :],
                            )

    nc.compile()
    return nc


def _get_nc(dtype=DTYPE):
    key = ("nc", dtype)
    if key not in _CACHE:
        _CACHE[key] = _build(dtype)
    return _CACHE[key]


def _shard_inputs(x, weight, bias, dtype=DTYPE):
    x = np.asarray(x, dtype=np.float32)
    weight = np.asarray(weight, dtype=np.float32)
    bias = np.asarray(bias, dtype=np.float32)
    if dtype == "bf16":
        import ml_dtypes

        np_io = ml_dtypes.bfloat16
    elif dtype == "fp16":
        np_io = np.float16
    else:
        np_io = np.float32
    in_maps = []
    for c in range(N_CORES):
        sl = slice(SPC * c, SPC * (c + 1))
        xs = np.ascontiguousarray(x[sl]).reshape(128, H, W).astype(np_io)
        # [s, co, ci, ky, kx] -> [s, ci, (ky kx), co], then block-diagonal
        wt = weight[sl].transpose(0, 2, 3, 4, 1).reshape(SPC, 32, 9, 32)
        ws = np.zeros((128, 9, 128), dtype=np_io)
        for s in range(SPC):
            ws[32 * s : 32 * (s + 1), :, 32 * s : 32 * (s + 1)] = wt[s]
        ws = ws.reshape(128, 9 * 128)
        bs = np.ascontiguousarray(bias[sl]).reshape(128, 1)
        in_maps.append({"x": xs, "w": ws, "bias_v": bs})
    return in_maps


def run(x, weight, bias, trace=False, dtype=DTYPE):
    from concourse.bass_utils import run_bass_kernel_spmd

    nc = _get_nc(dtype)
    in_maps = _shard_inputs(x, weight, bias, dtype)
    res = run_bass_kernel_spmd(
        nc, in_maps, core_ids=list(range(N_CORES)), trace=trace
    )
    out = np.empty((B, C_OUT, H, W), dtype=np.float32)
    for c in range(N_CORES):
        out[SPC * c : SPC * (c + 1)] = res.results[c]["out"].reshape(
            SPC, C_OUT, H, W
        )
    return out, res


def kernel(x, weight, bias):
    out, _ = run(x, weight, bias, trace=False)
    return out

